# revision 7
# baseline (speedup 1.0000x reference)
"""Multi-head causal attention (b=2, n=2048, dim=1024, h=16, d=64) on 8 TRN2
NeuronCores.

Sharding: core c handles batch b = c//4 and head-group g = c%4 (4 heads of 64
dims each).  Attention is independent per (b, h), so there is no cross-device
communication: each core computes its head-group's partial output-projection
(rank-256 contribution to out @ Wo) and the host sums the 4 partials per batch
and adds bo.

Per-core dataflow (all matmul inputs bf16, fp32 PSUM accumulation):
  - host supplies x[b].T so the contraction dim (model dim) lands on SBUF
    partitions with no on-device transposes anywhere.
  - qT/kT [hd, n] = Wq/Wk.T @ x.T   (lhsT = W slice, rhs = xT)      [PE]
  - V [n, hd] natural               (lhsT = xT slice, rhs = Wv)     [PE]
  - scores S^T[j, i] per head       (lhsT = kT slice, rhs = qT)     [PE]
  - P = exp(S^T/8), bf16            (fused scale, PSUM->SBUF)       [ACT]
  - causal mask on diagonal tiles   (memset + upper-tri multiply)   [DVE]
  - attnV with a ones-column on V: out rows 0..63 = V.T @ P,
    row 64 = softmax denominators   (one fused matmul chain)        [PE]
  - normalize by broadcast reciprocal of the denominator row        [DVE+GPSIMD]
  - partial out-projection          (lhsT = stacked outT, rhs = Wo) [PE]
"""

from contextlib import ExitStack

import numpy as np
import ml_dtypes

import concourse.bass as bass
import concourse.mybir as mybir
from concourse import bacc
import concourse.tile as tile
from concourse import library_config
from concourse.bass_utils import run_bass_kernel_spmd

BF16 = ml_dtypes.bfloat16
bf16 = mybir.dt.bfloat16
f32 = mybir.dt.float32

B, N, DIM = 2, 2048, 1024
HEADS, D = 16, 64
NCORES = 8
NH = 4                    # heads per core
HD = NH * D               # 256 head-dims per core
SCALE = D ** -0.5         # 0.125


def _emit(tc, xT, wq, wk, wv, wo, bq2, bk2, bv, tri, out, n, dim):
    nc = tc.nc
    KT = dim // 128       # k-tiles over model dim
    JT = n // 128         # j-tiles over sequence
    MB = n // 512         # i-blocks over sequence
    EXP = mybir.ActivationFunctionType.Exp

    with ExitStack() as ctx:
        cpool = ctx.enter_context(tc.tile_pool(name="consts", bufs=1))
        ppool = ctx.enter_context(tc.tile_pool(name="ptiles", bufs=12))
        wpool = ctx.enter_context(tc.tile_pool(name="work", bufs=3))
        ps2 = ctx.enter_context(tc.tile_pool(name="ps2", bufs=2, space="PSUM"))
        ps1 = ctx.enter_context(tc.tile_pool(name="ps1", bufs=4, space="PSUM"))
        dpool = ctx.enter_context(tc.tile_pool(name="dscratch", bufs=3, space="DRAM"))

        # ---- constant / persistent tiles + loads ----
        xt = cpool.tile([128, KT, n], bf16)
        for kt in range(KT):
            nc.sync.dma_start(out=xt[:, kt, :], in_=xT[kt * 128:(kt + 1) * 128, :])
        wq_sb = cpool.tile([128, KT, HD], bf16)
        wk_sb = cpool.tile([128, KT, HD], bf16)
        wv_sb = cpool.tile([128, KT, HD], bf16)
        for kt in range(KT):
            nc.sync.dma_start(out=wq_sb[:, kt, :], in_=wq[kt * 128:(kt + 1) * 128, :])
            nc.sync.dma_start(out=wk_sb[:, kt, :], in_=wk[kt * 128:(kt + 1) * 128, :])
            nc.sync.dma_start(out=wv_sb[:, kt, :], in_=wv[kt * 128:(kt + 1) * 128, :])
        wo_sb = cpool.tile([128, 2, dim], bf16)
        for kt2 in range(2):
            nc.sync.dma_start(out=wo_sb[:, kt2, :], in_=wo[kt2 * 128:(kt2 + 1) * 128, :])
        bq_sb = cpool.tile([128, 2], f32)
        nc.sync.dma_start(out=bq_sb, in_=bq2)
        bk_sb = cpool.tile([128, 2], f32)
        nc.sync.dma_start(out=bk_sb, in_=bk2)
        bvb = cpool.tile([128, HD], f32)
        nc.gpsimd.dma_start(out=bvb, in_=bv.to_broadcast([128, HD]))
        tri_sb = cpool.tile([128, 128], bf16)
        nc.sync.dma_start(out=tri_sb, in_=tri)

        qt_sb = cpool.tile([128, 2, n], bf16)
        kt_sb = cpool.tile([128, 2, n], bf16)
        v_sb = cpool.tile([128, JT, NH, D + 1], bf16)
        ot_sb = cpool.tile([128, 2, n], bf16)
        nc.vector.memset(v_sb[:, :, :, D:D + 1], 1.0)

        # ---- phase 1: QKV projections ----
        for mt in range(2):                       # hd M-tiles of 128
            for s in range(n // 1024):            # paired 512-col chunks
                for w_sb, b_sb, dst in ((wq_sb, bq_sb, qt_sb), (wk_sb, bk_sb, kt_sb)):
                    ps = ps2.tile([128, 1024], f32, tag="ps2")
                    for half in range(2):
                        col = s * 1024 + half * 512
                        for kt in range(KT):
                            nc.tensor.matmul(
                                ps[:, half * 512:(half + 1) * 512],
                                w_sb[:, kt, mt * 128:(mt + 1) * 128],
                                xt[:, kt, col:col + 512],
                                start=(kt == 0), stop=(kt == KT - 1))
                    nc.vector.tensor_scalar_add(
                        dst[:, mt, s * 1024:(s + 1) * 1024], ps, b_sb[:, mt:mt + 1])
        for jt in range(JT):
            ps = ps1.tile([128, 512], f32, tag="ps1")
            for kt in range(KT):
                nc.tensor.matmul(
                    ps[:, 0:HD],
                    xt[:, kt, jt * 128:(jt + 1) * 128],
                    wv_sb[:, kt, :],
                    start=(kt == 0), stop=(kt == KT - 1))
            nc.vector.tensor_add(
                v_sb[:, jt, :, 0:D],
                ps[:, 0:HD].rearrange("p (h d) -> p h d", h=NH),
                bvb.rearrange("p (h d) -> p h d", h=NH))

        # ---- phase 2: attention + output projection, streamed per i-block ----
        for m in range(MB):
            i0 = m * 512
            njt = 4 * m + 4                       # causal j-tiles for this block
            for h in range(NH):
                pair, hh = divmod(h, 2)
                r0, r1 = hh * 64, (hh + 1) * 64
                p_tiles = []
                for jp in range(njt // 2):        # j-tile pairs share a PSUM pair
                    ps = ps2.tile([128, 1024], f32, tag="ps2")
                    for half in range(2):
                        jt = 2 * jp + half
                        nc.tensor.matmul(
                            ps[:, half * 512:(half + 1) * 512],
                            kt_sb[r0:r1, pair, jt * 128:(jt + 1) * 128],
                            qt_sb[r0:r1, pair, i0:i0 + 512],
                            start=True, stop=True)
                    p = ppool.tile([128, 1024], bf16, tag="p")
                    nc.scalar.activation(out=p, in_=ps, func=EXP, scale=SCALE)
                    for half in range(2):
                        jt = 2 * jp + half
                        r = jt - 4 * m            # diagonal-region index
                        if r >= 0:
                            c0 = half * 512 + 128 * r
                            if r > 0:
                                nc.vector.memset(p[:, half * 512:c0], 0.0)
                            nc.vector.tensor_mul(
                                p[:, c0:c0 + 128], p[:, c0:c0 + 128], tri_sb)
                    p_tiles.append(p)
                o_ps = ps1.tile([128, 512], f32, tag="ps1")
                for jt in range(njt):
                    jp, half = divmod(jt, 2)
                    nc.tensor.matmul(
                        o_ps[0:D + 1, :],
                        v_sb[:, jt, h, :],
                        p_tiles[jp][:, half * 512:(half + 1) * 512],
                        start=(jt == 0), stop=(jt == njt - 1))
                rrow = wpool.tile([1, 512], f32)
                nc.vector.reciprocal(rrow, o_ps[D:D + 1, :])
                # broadcast across partitions via a DRAM roundtrip (partition-
                # step-0 reads are only legal from DRAM)
                rd = dpool.tile([1, 512], f32)
                nc.sync.dma_start(out=rd, in_=rrow)
                bc = wpool.tile([64, 512], f32)
                nc.gpsimd.dma_start(out=bc, in_=rd.to_broadcast([64, 512]))
                if hh == 0:
                    nc.vector.tensor_mul(ot_sb[0:64, pair, i0:i0 + 512],
                                         o_ps[0:64, :], bc)
                else:
                    tmp = wpool.tile([64, 512], bf16)
                    nc.vector.tensor_mul(tmp, o_ps[0:64, :], bc)
                    nc.sync.dma_start(out=ot_sb[64:128, pair, i0:i0 + 512], in_=tmp)
            for nt in range(4 * m, 4 * m + 4):    # output projection, this i-block
                for c2 in range(dim // 512):
                    f_ps = ps1.tile([128, 512], f32, tag="ps1")
                    for kt2 in range(2):
                        nc.tensor.matmul(
                            f_ps,
                            ot_sb[:, kt2, nt * 128:(nt + 1) * 128],
                            wo_sb[:, kt2, c2 * 512:(c2 + 1) * 512],
                            start=(kt2 == 0), stop=(kt2 == 1))
                    osb = wpool.tile([128, 512], f32, bufs=4)
                    nc.vector.tensor_copy(osb, f_ps)
                    nc.sync.dma_start(
                        out=out[nt * 128:(nt + 1) * 128, c2 * 512:(c2 + 1) * 512],
                        in_=osb)


def build(n=N, dim=DIM):
    nc = bacc.Bacc("TRN2")
    xT = nc.dram_tensor("xT", [dim, n], bf16, kind="ExternalInput")
    wq = nc.dram_tensor("wq", [dim, HD], bf16, kind="ExternalInput")
    wk = nc.dram_tensor("wk", [dim, HD], bf16, kind="ExternalInput")
    wv = nc.dram_tensor("wv", [dim, HD], bf16, kind="ExternalInput")
    wo = nc.dram_tensor("wo", [HD, dim], bf16, kind="ExternalInput")
    bq2 = nc.dram_tensor("bq2", [128, 2], f32, kind="ExternalInput")
    bk2 = nc.dram_tensor("bk2", [128, 2], f32, kind="ExternalInput")
    bv = nc.dram_tensor("bv", [1, HD], f32, kind="ExternalInput")
    tri = nc.dram_tensor("tri", [128, 128], bf16, kind="ExternalInput")
    out = nc.dram_tensor("out", [n, dim], f32, kind="ExternalOutput")
    with tile.TileContext(nc) as tc:
        _emit(tc, xT.ap(), wq.ap(), wk.ap(), wv.ap(), wo.ap(), bq2.ap(),
              bk2.ap(), bv.ap(), tri.ap(), out.ap(), n, dim)
    nc.finalize()
    return nc


_NC = None


def _get_nc():
    global _NC
    if _NC is None:
        _NC = build()
    return _NC


def make_in_maps(x, Wq, bq, Wkv, bkv, Wo):
    tri = np.triu(np.ones((128, 128), np.float32)).astype(BF16)
    xts = [np.ascontiguousarray(x[b].T).astype(BF16) for b in range(B)]
    in_maps = []
    for c in range(NCORES):
        b, g = divmod(c, NCORES // B)
        cs = slice(HD * g, HD * (g + 1))
        in_maps.append({
            "xT": xts[b],
            "wq": np.ascontiguousarray(Wq[:, cs]).astype(BF16),
            "wk": np.ascontiguousarray(Wkv[:, HD * g:HD * (g + 1)]).astype(BF16),
            "wv": np.ascontiguousarray(Wkv[:, DIM + HD * g:DIM + HD * (g + 1)]).astype(BF16),
            "wo": np.ascontiguousarray(Wo[cs, :]).astype(BF16),
            "bq2": np.ascontiguousarray(bq[cs].reshape(2, 128).T).astype(np.float32),
            "bk2": np.ascontiguousarray(bkv[HD * g:HD * (g + 1)].reshape(2, 128).T).astype(np.float32),
            "bv": np.ascontiguousarray(bkv[DIM + HD * g:DIM + HD * (g + 1)].reshape(1, HD)).astype(np.float32),
            "tri": tri,
        })
    return in_maps


def _run(x, Wq, bq, Wkv, bkv, Wo, bo, **spmd_kwargs):
    x = np.asarray(x, np.float32)
    Wq = np.asarray(Wq, np.float32)
    bq = np.asarray(bq, np.float32)
    Wkv = np.asarray(Wkv, np.float32)
    bkv = np.asarray(bkv, np.float32)
    Wo = np.asarray(Wo, np.float32)
    bo = np.asarray(bo, np.float32)
    nc = _get_nc()
    in_maps = make_in_maps(x, Wq, bq, Wkv, bkv, Wo)
    res = run_bass_kernel_spmd(nc, in_maps, core_ids=list(range(NCORES)),
                               **spmd_kwargs)
    g = NCORES // B
    y = np.empty((B, N, DIM), np.float32)
    for b in range(B):
        acc = res.results[g * b]["out"].astype(np.float32)
        for i in range(1, g):
            acc = acc + res.results[g * b + i]["out"]
        y[b] = acc + bo
    return y, res


def kernel(x, Wq, bq, Wkv, bkv, Wo, bo):
    return _run(x, Wq, bq, Wkv, bkv, Wo, bo)[0]


# revision 10
# speedup vs baseline: 1.0454x; 1.0454x over previous
"""Multi-head causal attention (b=2, n=2048, dim=1024, h=16, d=64) on 8 TRN2
NeuronCores.

Sharding: core c handles batch b = c//4 and head-group g = c%4 (4 heads of 64
dims each).  Attention is independent per (b, h), so there is no cross-device
communication: each core computes its head-group's partial output-projection
(rank-256 contribution to out @ Wo) and the host sums the 4 partials per batch
and adds bo.

Per-core dataflow (all matmul inputs bf16, fp32 PSUM accumulation):
  - host supplies x[b].T so the contraction dim (model dim) lands on SBUF
    partitions with no on-device transposes anywhere.
  - qT/kT [hd, n] = Wq/Wk.T @ x.T   (lhsT = W slice, rhs = xT)      [PE]
  - V [n, hd] natural               (lhsT = xT slice, rhs = Wv)     [PE]
  - scores S^T[j, i] per head       (lhsT = kT slice, rhs = qT)     [PE]
  - P = exp(S^T/8), bf16            (fused scale, PSUM->SBUF)       [ACT]
  - causal mask on diagonal tiles   (memset + upper-tri multiply)   [DVE]
  - attnV with a ones-column on V: out rows 0..63 = V.T @ P,
    row 64 = softmax denominators   (one fused matmul chain)        [PE]
  - normalize by broadcast reciprocal of the denominator row        [DVE+GPSIMD]
  - partial out-projection          (lhsT = stacked outT, rhs = Wo) [PE]
"""

from contextlib import ExitStack

import numpy as np
import ml_dtypes

import concourse.bass as bass
import concourse.mybir as mybir
from concourse import bacc
import concourse.tile as tile
from concourse import library_config
from concourse.bass_utils import run_bass_kernel_spmd

BF16 = ml_dtypes.bfloat16
bf16 = mybir.dt.bfloat16
f32 = mybir.dt.float32

B, N, DIM = 2, 2048, 1024
HEADS, D = 16, 64
NCORES = 8
NH = 4                    # heads per core
HD = NH * D               # 256 head-dims per core
SCALE = D ** -0.5         # 0.125


def _emit(tc, xT, wq, wk, wv, wo, bq2, bk2, bv, tri, out, n, dim):
    nc = tc.nc
    KT = dim // 128       # k-tiles over model dim
    JT = n // 128         # j-tiles over sequence
    MB = n // 512         # i-blocks over sequence
    EXP = mybir.ActivationFunctionType.Exp

    with ExitStack() as ctx:
        cpool = ctx.enter_context(tc.tile_pool(name="consts", bufs=1))
        ppool = ctx.enter_context(tc.tile_pool(name="ptiles", bufs=18))
        wpool = ctx.enter_context(tc.tile_pool(name="work", bufs=3))
        ps2 = ctx.enter_context(tc.tile_pool(name="ps2", bufs=3, space="PSUM"))
        ps1 = ctx.enter_context(tc.tile_pool(name="ps1", bufs=2, space="PSUM"))
        dpool = ctx.enter_context(tc.tile_pool(name="dscratch", bufs=4, space="DRAM"))

        # ---- constant / persistent tiles + loads ----
        xt = cpool.tile([128, KT, n], bf16)
        for kt in range(KT):
            nc.sync.dma_start(out=xt[:, kt, :], in_=xT[kt * 128:(kt + 1) * 128, :])
        wq_sb = cpool.tile([128, KT, HD], bf16)
        wk_sb = cpool.tile([128, KT, HD], bf16)
        wv_sb = cpool.tile([128, KT, HD], bf16)
        for kt in range(KT):
            nc.sync.dma_start(out=wq_sb[:, kt, :], in_=wq[kt * 128:(kt + 1) * 128, :])
            nc.sync.dma_start(out=wk_sb[:, kt, :], in_=wk[kt * 128:(kt + 1) * 128, :])
            nc.sync.dma_start(out=wv_sb[:, kt, :], in_=wv[kt * 128:(kt + 1) * 128, :])
        wo_sb = cpool.tile([128, 2, dim], bf16)
        for kt2 in range(2):
            nc.sync.dma_start(out=wo_sb[:, kt2, :], in_=wo[kt2 * 128:(kt2 + 1) * 128, :])
        bq_sb = cpool.tile([128, 2], f32)
        nc.sync.dma_start(out=bq_sb, in_=bq2)
        bk_sb = cpool.tile([128, 2], f32)
        nc.sync.dma_start(out=bk_sb, in_=bk2)
        bvb = cpool.tile([128, HD], f32)
        nc.gpsimd.dma_start(out=bvb, in_=bv.to_broadcast([128, HD]))
        tri_sb = cpool.tile([128, 128], bf16)
        nc.sync.dma_start(out=tri_sb, in_=tri)

        qt_sb = cpool.tile([128, 2, n], bf16)
        kt_sb = cpool.tile([128, 2, n], bf16)
        v_sb = cpool.tile([128, JT, NH, D + 1], bf16)
        ot_sb = cpool.tile([128, 2, n], bf16)
        nc.vector.memset(v_sb[:, :, :, D:D + 1], 1.0)

        # ---- phase 1: QKV projections ----
        for mt in range(2):                       # hd M-tiles of 128
            for s in range(n // 1024):            # paired 512-col chunks
                for w_sb, b_sb, dst in ((wq_sb, bq_sb, qt_sb), (wk_sb, bk_sb, kt_sb)):
                    ps = ps2.tile([128, 1024], f32, tag="ps2")
                    for half in range(2):
                        col = s * 1024 + half * 512
                        for kt in range(KT):
                            nc.tensor.matmul(
                                ps[:, half * 512:(half + 1) * 512],
                                w_sb[:, kt, mt * 128:(mt + 1) * 128],
                                xt[:, kt, col:col + 512],
                                start=(kt == 0), stop=(kt == KT - 1))
                    nc.vector.tensor_scalar_add(
                        dst[:, mt, s * 1024:(s + 1) * 1024], ps, b_sb[:, mt:mt + 1])
        for jt in range(JT):
            ps = ps1.tile([128, 512], f32, tag="ps1")
            for kt in range(KT):
                nc.tensor.matmul(
                    ps[:, 0:HD],
                    xt[:, kt, jt * 128:(jt + 1) * 128],
                    wv_sb[:, kt, :],
                    start=(kt == 0), stop=(kt == KT - 1))
            nc.vector.tensor_add(
                v_sb[:, jt, :, 0:D],
                ps[:, 0:HD].rearrange("p (h d) -> p h d", h=NH),
                bvb.rearrange("p (h d) -> p h d", h=NH))

        # ---- phase 2: attention + output projection, streamed per i-block ----
        for m in range(MB):
            i0 = m * 512
            njt = 4 * m + 4                       # causal j-tiles for this block
            for pair in range(2):                 # head pairs (hh alternates ->
                p_tiles = {0: [], 1: []}          #  LDW ping-pongs array halves)
                for jp in range(njt // 2):        # j-tile pairs share a PSUM pair
                    for hh in range(2):
                        r0, r1 = hh * 64, (hh + 1) * 64
                        ps = ps2.tile([128, 1024], f32, tag="ps2")
                        for half in range(2):
                            jt = 2 * jp + half
                            nc.tensor.matmul(
                                ps[:, half * 512:(half + 1) * 512],
                                kt_sb[r0:r1, pair, jt * 128:(jt + 1) * 128],
                                qt_sb[r0:r1, pair, i0:i0 + 512],
                                start=True, stop=True)
                        p = ppool.tile([128, 1024], bf16, tag="p")
                        # the all-diagonal pair's first valid column is 256
                        c0 = 256 if 2 * jp == 4 * m + 2 else 0
                        nc.scalar.activation(out=p[:, c0:], in_=ps[:, c0:],
                                             func=EXP, scale=SCALE)
                        for half in range(2):
                            jt = 2 * jp + half
                            r = jt - 4 * m        # diagonal-region index
                            if r >= 0:
                                cm = half * 512 + 128 * r
                                if r > 0:
                                    nc.vector.memset(p[:, half * 512:cm], 0.0)
                                nc.vector.tensor_mul(
                                    p[:, cm:cm + 128], p[:, cm:cm + 128], tri_sb)
                        p_tiles[hh].append(p)
                for hh in range(2):
                    o_ps = ps1.tile([128, 512], f32, tag="ps1")
                    for jt in range(njt):
                        jp, half = divmod(jt, 2)
                        nc.tensor.matmul(
                            o_ps[0:D + 1, :],
                            v_sb[:, jt, 2 * pair + hh, :],
                            p_tiles[hh][jp][:, half * 512:(half + 1) * 512],
                            start=(jt == 0), stop=(jt == njt - 1))
                    # softmax denominators: copy out, reshape through DRAM onto
                    # 128 partitions (single-partition RECIPROCAL is 8 cyc/elem),
                    # take reciprocals, then broadcast back over 64 partitions
                    # (partition-step-0 reads are only legal from DRAM).
                    srow = wpool.tile([1, 512], f32)
                    nc.vector.tensor_copy(srow, o_ps[D:D + 1, :])
                    rd = dpool.tile([1, 512], f32)
                    nc.sync.dma_start(out=rd, in_=srow)
                    s4 = wpool.tile([128, 4], f32)
                    nc.sync.dma_start(out=s4, in_=rd.rearrange("o (p c) -> (o p) c", p=128))
                    nc.vector.reciprocal(s4, s4)
                    rd2 = dpool.tile([1, 512], f32)
                    nc.sync.dma_start(out=rd2.rearrange("o (p c) -> (o p) c", p=128), in_=s4)
                    bc = wpool.tile([64, 512], f32)
                    nc.gpsimd.dma_start(out=bc, in_=rd2.to_broadcast([64, 512]))
                    if hh == 0:
                        nc.vector.tensor_mul(ot_sb[0:64, pair, i0:i0 + 512],
                                             o_ps[0:64, :], bc)
                    else:
                        tmp = wpool.tile([64, 512], bf16)
                        nc.vector.tensor_mul(tmp, o_ps[0:64, :], bc)
                        nc.sync.dma_start(out=ot_sb[64:128, pair, i0:i0 + 512], in_=tmp)
            for nt in range(4 * m, 4 * m + 4):    # output projection, this i-block
                for c2 in range(dim // 512):
                    f_ps = ps1.tile([128, 512], f32, tag="ps1")
                    for kt2 in range(2):
                        nc.tensor.matmul(
                            f_ps,
                            ot_sb[:, kt2, nt * 128:(nt + 1) * 128],
                            wo_sb[:, kt2, c2 * 512:(c2 + 1) * 512],
                            start=(kt2 == 0), stop=(kt2 == 1))
                    osb = wpool.tile([128, 512], f32, bufs=4)
                    nc.vector.tensor_copy(osb, f_ps)
                    nc.sync.dma_start(
                        out=out[nt * 128:(nt + 1) * 128, c2 * 512:(c2 + 1) * 512],
                        in_=osb)


def build(n=N, dim=DIM):
    nc = bacc.Bacc("TRN2")
    xT = nc.dram_tensor("xT", [dim, n], bf16, kind="ExternalInput")
    wq = nc.dram_tensor("wq", [dim, HD], bf16, kind="ExternalInput")
    wk = nc.dram_tensor("wk", [dim, HD], bf16, kind="ExternalInput")
    wv = nc.dram_tensor("wv", [dim, HD], bf16, kind="ExternalInput")
    wo = nc.dram_tensor("wo", [HD, dim], bf16, kind="ExternalInput")
    bq2 = nc.dram_tensor("bq2", [128, 2], f32, kind="ExternalInput")
    bk2 = nc.dram_tensor("bk2", [128, 2], f32, kind="ExternalInput")
    bv = nc.dram_tensor("bv", [1, HD], f32, kind="ExternalInput")
    tri = nc.dram_tensor("tri", [128, 128], bf16, kind="ExternalInput")
    out = nc.dram_tensor("out", [n, dim], f32, kind="ExternalOutput")
    with tile.TileContext(nc) as tc:
        _emit(tc, xT.ap(), wq.ap(), wk.ap(), wv.ap(), wo.ap(), bq2.ap(),
              bk2.ap(), bv.ap(), tri.ap(), out.ap(), n, dim)
    nc.finalize()
    return nc


_NC = None


def _get_nc():
    global _NC
    if _NC is None:
        _NC = build()
    return _NC


def make_in_maps(x, Wq, bq, Wkv, bkv, Wo):
    tri = np.triu(np.ones((128, 128), np.float32)).astype(BF16)
    xts = [np.ascontiguousarray(x[b].T).astype(BF16) for b in range(B)]
    in_maps = []
    for c in range(NCORES):
        b, g = divmod(c, NCORES // B)
        cs = slice(HD * g, HD * (g + 1))
        in_maps.append({
            "xT": xts[b],
            "wq": np.ascontiguousarray(Wq[:, cs]).astype(BF16),
            "wk": np.ascontiguousarray(Wkv[:, HD * g:HD * (g + 1)]).astype(BF16),
            "wv": np.ascontiguousarray(Wkv[:, DIM + HD * g:DIM + HD * (g + 1)]).astype(BF16),
            "wo": np.ascontiguousarray(Wo[cs, :]).astype(BF16),
            "bq2": np.ascontiguousarray(bq[cs].reshape(2, 128).T).astype(np.float32),
            "bk2": np.ascontiguousarray(bkv[HD * g:HD * (g + 1)].reshape(2, 128).T).astype(np.float32),
            "bv": np.ascontiguousarray(bkv[DIM + HD * g:DIM + HD * (g + 1)].reshape(1, HD)).astype(np.float32),
            "tri": tri,
        })
    return in_maps


def _run(x, Wq, bq, Wkv, bkv, Wo, bo, **spmd_kwargs):
    x = np.asarray(x, np.float32)
    Wq = np.asarray(Wq, np.float32)
    bq = np.asarray(bq, np.float32)
    Wkv = np.asarray(Wkv, np.float32)
    bkv = np.asarray(bkv, np.float32)
    Wo = np.asarray(Wo, np.float32)
    bo = np.asarray(bo, np.float32)
    nc = _get_nc()
    in_maps = make_in_maps(x, Wq, bq, Wkv, bkv, Wo)
    res = run_bass_kernel_spmd(nc, in_maps, core_ids=list(range(NCORES)),
                               **spmd_kwargs)
    g = NCORES // B
    y = np.empty((B, N, DIM), np.float32)
    for b in range(B):
        acc = res.results[g * b]["out"].astype(np.float32)
        for i in range(1, g):
            acc = acc + res.results[g * b + i]["out"]
        y[b] = acc + bo
    return y, res


def kernel(x, Wq, bq, Wkv, bkv, Wo, bo):
    return _run(x, Wq, bq, Wkv, bkv, Wo, bo)[0]


# revision 12
# speedup vs baseline: 1.1082x; 1.0600x over previous
"""Multi-head causal attention (b=2, n=2048, dim=1024, h=16, d=64) on 8 TRN2
NeuronCores.

Sharding: core c handles batch b = c//4 and head-group g = c%4 (4 heads of 64
dims each).  Attention is independent per (b, h), so there is no cross-device
communication: each core computes its head-group's partial output-projection
(rank-256 contribution to out @ Wo) and the host sums the 4 partials per batch
and adds bo.

Per-core dataflow (all matmul inputs bf16, fp32 PSUM accumulation):
  - host supplies x[b].T so the contraction dim (model dim) lands on SBUF
    partitions with no on-device transposes anywhere.
  - qT/kT [hd, n] = Wq/Wk.T @ x.T   (lhsT = W slice, rhs = xT)      [PE]
  - V [n, hd] natural               (lhsT = xT slice, rhs = Wv)     [PE]
  - scores S^T[j, i] per head       (lhsT = kT slice, rhs = qT)     [PE]
  - P = exp(S^T/8), bf16            (fused scale, PSUM->SBUF)       [ACT]
  - causal mask on diagonal tiles   (memset + upper-tri multiply)   [DVE]
  - attnV with a ones-column on V: out rows 0..63 = V.T @ P,
    row 64 = softmax denominators   (one fused matmul chain)        [PE]
  - normalize by broadcast reciprocal of the denominator row        [DVE+GPSIMD]
  - partial out-projection          (lhsT = stacked outT, rhs = Wo) [PE]
"""

from contextlib import ExitStack

import numpy as np
import ml_dtypes

import concourse.bass as bass
import concourse.mybir as mybir
from concourse import bacc
import concourse.tile as tile
from concourse import library_config
from concourse.bass_utils import run_bass_kernel_spmd

BF16 = ml_dtypes.bfloat16
bf16 = mybir.dt.bfloat16
f32 = mybir.dt.float32

B, N, DIM = 2, 2048, 1024
HEADS, D = 16, 64
NCORES = 8
NH = 4                    # heads per core
HD = NH * D               # 256 head-dims per core
SCALE = D ** -0.5         # 0.125


def _emit(tc, xT, wq, wk, wv, wo, bq2, bk2, bv, tri, out, n, dim):
    nc = tc.nc
    KT = dim // 128       # k-tiles over model dim
    JT = n // 128         # j-tiles over sequence
    MB = n // 512         # i-blocks over sequence
    EXP = mybir.ActivationFunctionType.Exp

    with ExitStack() as ctx:
        cpool = ctx.enter_context(tc.tile_pool(name="consts", bufs=1))
        ppool = ctx.enter_context(tc.tile_pool(name="ptiles", bufs=18))
        wpool = ctx.enter_context(tc.tile_pool(name="work", bufs=6))
        opool = ctx.enter_context(tc.tile_pool(name="otiles", bufs=3))
        ps2 = ctx.enter_context(tc.tile_pool(name="ps2", bufs=3, space="PSUM"))
        ps1 = ctx.enter_context(tc.tile_pool(name="ps1", bufs=2, space="PSUM"))
        dpool = ctx.enter_context(tc.tile_pool(name="dscratch", bufs=8, space="DRAM"))

        # ---- constant / persistent tiles + loads (weights first: they are
        # small and gate the first matmuls; xt streams behind them) ----
        wq_sb = cpool.tile([128, KT, HD], bf16)
        wk_sb = cpool.tile([128, KT, HD], bf16)
        wv_sb = cpool.tile([128, KT, HD], bf16)
        for kt in range(KT):
            nc.sync.dma_start(out=wq_sb[:, kt, :], in_=wq[kt * 128:(kt + 1) * 128, :])
            nc.sync.dma_start(out=wk_sb[:, kt, :], in_=wk[kt * 128:(kt + 1) * 128, :])
            nc.sync.dma_start(out=wv_sb[:, kt, :], in_=wv[kt * 128:(kt + 1) * 128, :])
        wo_sb = cpool.tile([128, 2, dim], bf16)
        for kt2 in range(2):
            nc.sync.dma_start(out=wo_sb[:, kt2, :], in_=wo[kt2 * 128:(kt2 + 1) * 128, :])
        bq_sb = cpool.tile([128, 2], f32)
        nc.sync.dma_start(out=bq_sb, in_=bq2)
        bk_sb = cpool.tile([128, 2], f32)
        nc.sync.dma_start(out=bk_sb, in_=bk2)
        bvb = cpool.tile([128, HD], f32)
        nc.gpsimd.dma_start(out=bvb, in_=bv.to_broadcast([128, HD]))
        tri_sb = cpool.tile([128, 128], bf16)
        nc.sync.dma_start(out=tri_sb, in_=tri)
        xt = cpool.tile([128, KT, n], bf16)
        for kt in range(KT):
            nc.sync.dma_start(out=xt[:, kt, :], in_=xT[kt * 128:(kt + 1) * 128, :])

        qt_sb = cpool.tile([128, 2, n], bf16)
        kt_sb = cpool.tile([128, 2, n], bf16)
        v_sb = cpool.tile([128, JT, NH, D + 1], bf16)
        nc.vector.memset(v_sb[:, :, :, D:D + 1], 1.0)

        # ---- phase 1: QKV projections ----
        for mt in range(2):                       # hd M-tiles of 128
            for s in range(n // 1024):            # paired 512-col chunks
                for w_sb, b_sb, dst in ((wq_sb, bq_sb, qt_sb), (wk_sb, bk_sb, kt_sb)):
                    ps = ps2.tile([128, 1024], f32, tag="ps2")
                    for half in range(2):
                        col = s * 1024 + half * 512
                        for kt in range(KT):
                            nc.tensor.matmul(
                                ps[:, half * 512:(half + 1) * 512],
                                w_sb[:, kt, mt * 128:(mt + 1) * 128],
                                xt[:, kt, col:col + 512],
                                start=(kt == 0), stop=(kt == KT - 1))
                    nc.vector.tensor_scalar_add(
                        dst[:, mt, s * 1024:(s + 1) * 1024], ps, b_sb[:, mt:mt + 1])
        for jt in range(JT):
            ps = ps1.tile([128, 512], f32, tag="ps1")
            for kt in range(KT):
                nc.tensor.matmul(
                    ps[:, 0:HD],
                    xt[:, kt, jt * 128:(jt + 1) * 128],
                    wv_sb[:, kt, :],
                    start=(kt == 0), stop=(kt == KT - 1))
            nc.vector.tensor_add(
                v_sb[:, jt, :, 0:D],
                ps[:, 0:HD].rearrange("p (h d) -> p h d", h=NH),
                bvb.rearrange("p (h d) -> p h d", h=NH))

        # ---- phase 2: attention + output projection, streamed per i-block ----
        for m in range(MB):
            i0 = m * 512
            njt = 4 * m + 4                       # causal j-tiles for this block
            ot_m = opool.tile([128, 2, 512], bf16)
            for pair in range(2):                 # head pairs (hh alternates ->
                p_tiles = {0: [], 1: []}          #  LDW ping-pongs array halves)
                for jp in range(njt // 2):        # j-tile pairs share a PSUM pair
                    for hh in range(2):
                        r0, r1 = hh * 64, (hh + 1) * 64
                        ps = ps2.tile([128, 1024], f32, tag="ps2")
                        for half in range(2):
                            jt = 2 * jp + half
                            nc.tensor.matmul(
                                ps[:, half * 512:(half + 1) * 512],
                                kt_sb[r0:r1, pair, jt * 128:(jt + 1) * 128],
                                qt_sb[r0:r1, pair, i0:i0 + 512],
                                start=True, stop=True)
                        p = ppool.tile([128, 1024], bf16, tag="p")
                        # the all-diagonal pair's first valid column is 256
                        c0 = 256 if 2 * jp == 4 * m + 2 else 0
                        nc.scalar.activation(out=p[:, c0:], in_=ps[:, c0:],
                                             func=EXP, scale=SCALE)
                        for half in range(2):
                            jt = 2 * jp + half
                            r = jt - 4 * m        # diagonal-region index
                            if r >= 0:
                                cm = half * 512 + 128 * r
                                if r > 0:
                                    nc.vector.memset(p[:, half * 512:cm], 0.0)
                                nc.vector.tensor_mul(
                                    p[:, cm:cm + 128], p[:, cm:cm + 128], tri_sb)
                        p_tiles[hh].append(p)
                for hh in range(2):
                    o_ps = ps1.tile([128, 512], f32, tag="ps1")
                    for jt in range(njt):
                        jp, half = divmod(jt, 2)
                        nc.tensor.matmul(
                            o_ps[0:D + 1, :],
                            v_sb[:, jt, 2 * pair + hh, :],
                            p_tiles[hh][jp][:, half * 512:(half + 1) * 512],
                            start=(jt == 0), stop=(jt == njt - 1))
                    # stage [out | denom] to SBUF immediately so the PSUM bank
                    # frees for the next attnV chain instead of being held
                    # through the normalization's DMA latency
                    u = wpool.tile([65, 512], f32)
                    nc.vector.tensor_copy(u, o_ps[0:D + 1, :])
                    # softmax denominators: reshape through DRAM onto 128
                    # partitions (single-partition RECIPROCAL is 8 cyc/elem),
                    # take reciprocals, then broadcast back over 64 partitions
                    # (partition-step-0 reads are only legal from DRAM).
                    rd = dpool.tile([1, 512], f32)
                    nc.sync.dma_start(out=rd, in_=u[D:D + 1, :])
                    s4 = wpool.tile([128, 4], f32)
                    nc.sync.dma_start(out=s4, in_=rd.rearrange("o (p c) -> (o p) c", p=128))
                    nc.vector.reciprocal(s4, s4)
                    rd2 = dpool.tile([1, 512], f32)
                    nc.sync.dma_start(out=rd2.rearrange("o (p c) -> (o p) c", p=128), in_=s4)
                    bc = wpool.tile([64, 512], f32)
                    nc.gpsimd.dma_start(out=bc, in_=rd2.to_broadcast([64, 512]))
                    if hh == 0:
                        nc.vector.tensor_mul(ot_m[0:64, pair, :], u[0:64, :], bc)
                    else:
                        tmp = wpool.tile([64, 512], bf16)
                        nc.vector.tensor_mul(tmp, u[0:64, :], bc)
                        nc.sync.dma_start(out=ot_m[64:128, pair, :], in_=tmp)
            for nt in range(4):                   # output projection, this i-block
                for c2 in range(dim // 512):
                    f_ps = ps1.tile([128, 512], f32, tag="ps1")
                    for kt2 in range(2):
                        nc.tensor.matmul(
                            f_ps,
                            ot_m[:, kt2, nt * 128:(nt + 1) * 128],
                            wo_sb[:, kt2, c2 * 512:(c2 + 1) * 512],
                            start=(kt2 == 0), stop=(kt2 == 1))
                    osb = wpool.tile([128, 512], f32, bufs=4)
                    nc.vector.tensor_copy(osb, f_ps)
                    gnt = 4 * m + nt
                    nc.sync.dma_start(
                        out=out[gnt * 128:(gnt + 1) * 128, c2 * 512:(c2 + 1) * 512],
                        in_=osb)


def build(n=N, dim=DIM):
    nc = bacc.Bacc("TRN2")
    xT = nc.dram_tensor("xT", [dim, n], bf16, kind="ExternalInput")
    wq = nc.dram_tensor("wq", [dim, HD], bf16, kind="ExternalInput")
    wk = nc.dram_tensor("wk", [dim, HD], bf16, kind="ExternalInput")
    wv = nc.dram_tensor("wv", [dim, HD], bf16, kind="ExternalInput")
    wo = nc.dram_tensor("wo", [HD, dim], bf16, kind="ExternalInput")
    bq2 = nc.dram_tensor("bq2", [128, 2], f32, kind="ExternalInput")
    bk2 = nc.dram_tensor("bk2", [128, 2], f32, kind="ExternalInput")
    bv = nc.dram_tensor("bv", [1, HD], f32, kind="ExternalInput")
    tri = nc.dram_tensor("tri", [128, 128], bf16, kind="ExternalInput")
    out = nc.dram_tensor("out", [n, dim], f32, kind="ExternalOutput")
    with tile.TileContext(nc) as tc:
        _emit(tc, xT.ap(), wq.ap(), wk.ap(), wv.ap(), wo.ap(), bq2.ap(),
              bk2.ap(), bv.ap(), tri.ap(), out.ap(), n, dim)
    nc.finalize()
    return nc


_NC = None


def _get_nc():
    global _NC
    if _NC is None:
        _NC = build()
    return _NC


def make_in_maps(x, Wq, bq, Wkv, bkv, Wo):
    tri = np.triu(np.ones((128, 128), np.float32)).astype(BF16)
    xts = [np.ascontiguousarray(x[b].T).astype(BF16) for b in range(B)]
    in_maps = []
    for c in range(NCORES):
        b, g = divmod(c, NCORES // B)
        cs = slice(HD * g, HD * (g + 1))
        in_maps.append({
            "xT": xts[b],
            "wq": np.ascontiguousarray(Wq[:, cs]).astype(BF16),
            "wk": np.ascontiguousarray(Wkv[:, HD * g:HD * (g + 1)]).astype(BF16),
            "wv": np.ascontiguousarray(Wkv[:, DIM + HD * g:DIM + HD * (g + 1)]).astype(BF16),
            "wo": np.ascontiguousarray(Wo[cs, :]).astype(BF16),
            "bq2": np.ascontiguousarray(bq[cs].reshape(2, 128).T).astype(np.float32),
            "bk2": np.ascontiguousarray(bkv[HD * g:HD * (g + 1)].reshape(2, 128).T).astype(np.float32),
            "bv": np.ascontiguousarray(bkv[DIM + HD * g:DIM + HD * (g + 1)].reshape(1, HD)).astype(np.float32),
            "tri": tri,
        })
    return in_maps


def _run(x, Wq, bq, Wkv, bkv, Wo, bo, **spmd_kwargs):
    x = np.asarray(x, np.float32)
    Wq = np.asarray(Wq, np.float32)
    bq = np.asarray(bq, np.float32)
    Wkv = np.asarray(Wkv, np.float32)
    bkv = np.asarray(bkv, np.float32)
    Wo = np.asarray(Wo, np.float32)
    bo = np.asarray(bo, np.float32)
    nc = _get_nc()
    in_maps = make_in_maps(x, Wq, bq, Wkv, bkv, Wo)
    res = run_bass_kernel_spmd(nc, in_maps, core_ids=list(range(NCORES)),
                               **spmd_kwargs)
    g = NCORES // B
    y = np.empty((B, N, DIM), np.float32)
    for b in range(B):
        acc = res.results[g * b]["out"].astype(np.float32)
        for i in range(1, g):
            acc = acc + res.results[g * b + i]["out"]
        y[b] = acc + bo
    return y, res


def kernel(x, Wq, bq, Wkv, bkv, Wo, bo):
    return _run(x, Wq, bq, Wkv, bkv, Wo, bo)[0]


# revision 13
# speedup vs baseline: 1.1726x; 1.0581x over previous
"""Multi-head causal attention (b=2, n=2048, dim=1024, h=16, d=64) on 8 TRN2
NeuronCores.

Sharding: core c handles batch b = c//4 and head-group g = c%4 (4 heads of 64
dims each).  Attention is independent per (b, h), so there is no cross-device
communication: each core computes its head-group's partial output-projection
(rank-256 contribution to out @ Wo) and the host sums the 4 partials per batch
and adds bo.

Per-core dataflow (all matmul inputs bf16, fp32 PSUM accumulation):
  - host supplies x[b].T so the contraction dim (model dim) lands on SBUF
    partitions with no on-device transposes anywhere.
  - qT/kT [hd, n] = Wq/Wk.T @ x.T   (lhsT = W slice, rhs = xT)      [PE]
  - V [n, hd] natural               (lhsT = xT slice, rhs = Wv)     [PE]
  - scores S^T[j, i] per head       (lhsT = kT slice, rhs = qT)     [PE]
  - P = exp(S^T/8), bf16            (fused scale, PSUM->SBUF)       [ACT]
  - causal mask on diagonal tiles   (memset + upper-tri multiply)   [DVE]
  - attnV with a ones-column on V: out rows 0..63 = V.T @ P,
    row 64 = softmax denominators   (one fused matmul chain)        [PE]
  - normalize by broadcast reciprocal of the denominator row        [DVE+GPSIMD]
  - partial out-projection          (lhsT = stacked outT, rhs = Wo) [PE]
"""

from contextlib import ExitStack

import numpy as np
import ml_dtypes

import concourse.bass as bass
import concourse.mybir as mybir
from concourse import bacc
import concourse.tile as tile
from concourse import library_config
from concourse.bass_utils import run_bass_kernel_spmd

BF16 = ml_dtypes.bfloat16
bf16 = mybir.dt.bfloat16
f32 = mybir.dt.float32

B, N, DIM = 2, 2048, 1024
HEADS, D = 16, 64
NCORES = 8
NH = 4                    # heads per core
HD = NH * D               # 256 head-dims per core
SCALE = D ** -0.5         # 0.125


def _emit(tc, xT, wq, wk, wv, wo, bq2, bk2, bv, tri, out, n, dim):
    nc = tc.nc
    KT = dim // 128       # k-tiles over model dim
    JT = n // 128         # j-tiles over sequence
    MB = n // 512         # i-blocks over sequence
    NS = n // 1024        # qkv column groups (1024 wide)
    EXP = mybir.ActivationFunctionType.Exp

    with ExitStack() as ctx:
        cpool = ctx.enter_context(tc.tile_pool(name="consts", bufs=1))
        ppool = ctx.enter_context(tc.tile_pool(name="ptiles", bufs=18))
        wpool = ctx.enter_context(tc.tile_pool(name="work", bufs=6))
        opool = ctx.enter_context(tc.tile_pool(name="otiles", bufs=3))
        ps2 = ctx.enter_context(tc.tile_pool(name="ps2", bufs=3, space="PSUM"))
        ps1 = ctx.enter_context(tc.tile_pool(name="ps1", bufs=2, space="PSUM"))
        dpool = ctx.enter_context(tc.tile_pool(name="dscratch", bufs=8, space="DRAM"))

        # ---- constant / persistent tiles; batched loads spread over the three
        # DMA rings (sync=xt, scalar=weights, gpsimd=small constants) so issue
        # serialization doesn't gate the first matmuls ----
        xt = cpool.tile([128, KT, n], bf16)
        nc.sync.dma_start(out=xt, in_=xT.rearrange("(kt p) n -> p kt n", p=128))
        wq_sb = cpool.tile([128, KT, HD], bf16)
        nc.scalar.dma_start(out=wq_sb, in_=wq.rearrange("(kt p) c -> p kt c", p=128))
        wk_sb = cpool.tile([128, KT, HD], bf16)
        nc.scalar.dma_start(out=wk_sb, in_=wk.rearrange("(kt p) c -> p kt c", p=128))
        wv_sb = cpool.tile([128, KT, HD], bf16)
        nc.scalar.dma_start(out=wv_sb, in_=wv.rearrange("(kt p) c -> p kt c", p=128))
        wo_sb = cpool.tile([128, 2, dim], bf16)
        nc.scalar.dma_start(out=wo_sb, in_=wo.rearrange("(kt p) c -> p kt c", p=128))
        bq_sb = cpool.tile([128, 2], f32)
        nc.gpsimd.dma_start(out=bq_sb, in_=bq2)
        bk_sb = cpool.tile([128, 2], f32)
        nc.gpsimd.dma_start(out=bk_sb, in_=bk2)
        bvb = cpool.tile([128, HD], f32)
        nc.gpsimd.dma_start(out=bvb, in_=bv.to_broadcast([128, HD]))
        tri_sb = cpool.tile([128, 128], bf16)
        nc.gpsimd.dma_start(out=tri_sb, in_=tri)

        qt_sb = cpool.tile([128, 2, n], bf16)
        kt_sb = cpool.tile([128, 2, n], bf16)
        v_sb = cpool.tile([128, JT, NH, D + 1], bf16)
        nc.vector.memset(v_sb[:, :, :, D:D + 1], 1.0)

        def qkv_group(s):
            """Q/K projections for column group s + V for its j-tiles."""
            for mt in range(2):                   # hd M-tiles of 128
                for w_sb, b_sb, dst in ((wq_sb, bq_sb, qt_sb), (wk_sb, bk_sb, kt_sb)):
                    ps = ps2.tile([128, 1024], f32, tag="ps2")
                    for half in range(2):
                        col = s * 1024 + half * 512
                        for kt in range(KT):
                            nc.tensor.matmul(
                                ps[:, half * 512:(half + 1) * 512],
                                w_sb[:, kt, mt * 128:(mt + 1) * 128],
                                xt[:, kt, col:col + 512],
                                start=(kt == 0), stop=(kt == KT - 1))
                    nc.vector.tensor_scalar_add(
                        dst[:, mt, s * 1024:(s + 1) * 1024], ps, b_sb[:, mt:mt + 1])
            for jt in range(8 * s, 8 * s + 8):
                ps = ps1.tile([128, 512], f32, tag="ps1")
                for kt in range(KT):
                    nc.tensor.matmul(
                        ps[:, 0:HD],
                        xt[:, kt, jt * 128:(jt + 1) * 128],
                        wv_sb[:, kt, :],
                        start=(kt == 0), stop=(kt == KT - 1))
                nc.vector.tensor_add(
                    v_sb[:, jt, :, 0:D],
                    ps[:, 0:HD].rearrange("p (h d) -> p h d", h=NH),
                    bvb.rearrange("p (h d) -> p h d", h=NH))

        def attn_block(m):
            """Attention + partial output projection for i-block m."""
            i0 = m * 512
            njt = 4 * m + 4                       # causal j-tiles for this block
            ot_m = opool.tile([128, 2, 512], bf16, tag="ot")
            for pair in range(2):                 # head pairs (hh alternates ->
                p_tiles = {0: [], 1: []}          #  LDW ping-pongs array halves)
                for jp in range(njt // 2):        # j-tile pairs share a PSUM pair
                    for hh in range(2):
                        r0, r1 = hh * 64, (hh + 1) * 64
                        ps = ps2.tile([128, 1024], f32, tag="ps2")
                        for half in range(2):
                            jt = 2 * jp + half
                            nc.tensor.matmul(
                                ps[:, half * 512:(half + 1) * 512],
                                kt_sb[r0:r1, pair, jt * 128:(jt + 1) * 128],
                                qt_sb[r0:r1, pair, i0:i0 + 512],
                                start=True, stop=True)
                        p = ppool.tile([128, 1024], bf16, tag="p")
                        # the all-diagonal pair's first valid column is 256
                        c0 = 256 if 2 * jp == 4 * m + 2 else 0
                        nc.scalar.activation(out=p[:, c0:], in_=ps[:, c0:],
                                             func=EXP, scale=SCALE)
                        for half in range(2):
                            jt = 2 * jp + half
                            r = jt - 4 * m        # diagonal-region index
                            if r >= 0:
                                cm = half * 512 + 128 * r
                                if r > 0:
                                    nc.vector.memset(p[:, half * 512:cm], 0.0)
                                nc.vector.tensor_mul(
                                    p[:, cm:cm + 128], p[:, cm:cm + 128], tri_sb)
                        p_tiles[hh].append(p)
                for hh in range(2):
                    o_ps = ps1.tile([128, 512], f32, tag="ps1")
                    for jt in range(njt):
                        jp, half = divmod(jt, 2)
                        nc.tensor.matmul(
                            o_ps[0:D + 1, :],
                            v_sb[:, jt, 2 * pair + hh, :],
                            p_tiles[hh][jp][:, half * 512:(half + 1) * 512],
                            start=(jt == 0), stop=(jt == njt - 1))
                    # stage [out | denom] to SBUF immediately so the PSUM bank
                    # frees for the next attnV chain instead of being held
                    # through the normalization's DMA latency
                    u = wpool.tile([65, 512], f32)
                    nc.vector.tensor_copy(u, o_ps[0:D + 1, :])
                    # softmax denominators: reshape through DRAM onto 128
                    # partitions (single-partition RECIPROCAL is 8 cyc/elem),
                    # take reciprocals, then broadcast back over 64 partitions
                    # (partition-step-0 reads are only legal from DRAM).
                    rd = dpool.tile([1, 512], f32)
                    nc.sync.dma_start(out=rd, in_=u[D:D + 1, :])
                    s4 = wpool.tile([128, 4], f32)
                    nc.sync.dma_start(out=s4, in_=rd.rearrange("o (p c) -> (o p) c", p=128))
                    nc.vector.reciprocal(s4, s4)
                    rd2 = dpool.tile([1, 512], f32)
                    nc.sync.dma_start(out=rd2.rearrange("o (p c) -> (o p) c", p=128), in_=s4)
                    bc = wpool.tile([64, 512], f32)
                    nc.gpsimd.dma_start(out=bc, in_=rd2.to_broadcast([64, 512]))
                    # normalized write straight into the stacked tile; odd heads
                    # use a partition-shifted DVE write (rows 64..127)
                    nc.vector.tensor_mul(ot_m[hh * 64:hh * 64 + 64, pair, :],
                                         u[0:64, :], bc)
            for nt in range(4):                   # output projection, this i-block
                for c2 in range(dim // 512):
                    f_ps = ps1.tile([128, 512], f32, tag="ps1")
                    for kt2 in range(2):
                        nc.tensor.matmul(
                            f_ps,
                            ot_m[:, kt2, nt * 128:(nt + 1) * 128],
                            wo_sb[:, kt2, c2 * 512:(c2 + 1) * 512],
                            start=(kt2 == 0), stop=(kt2 == 1))
                    osb = wpool.tile([128, 512], f32, bufs=4)
                    nc.vector.tensor_copy(osb, f_ps)
                    gnt = 4 * m + nt
                    nc.sync.dma_start(
                        out=out[gnt * 128:(gnt + 1) * 128, c2 * 512:(c2 + 1) * 512],
                        in_=osb)

        # Interleave qkv groups with attention blocks: the attention phase is
        # ACT(exp)-gated, so the next qkv group's matmuls fill PE gaps; the
        # smallest block (m=0) goes last to minimize the tail.
        if NS == 1:
            qkv_group(0)
            for m in range(MB - 1, -1, -1):
                attn_block(m)
        else:
            qkv_group(0)
            attn_block(1)
            for s in range(1, NS):
                qkv_group(s)
            attn_block(3)
            attn_block(2)
            attn_block(0)


def build(n=N, dim=DIM):
    nc = bacc.Bacc("TRN2")
    xT = nc.dram_tensor("xT", [dim, n], bf16, kind="ExternalInput")
    wq = nc.dram_tensor("wq", [dim, HD], bf16, kind="ExternalInput")
    wk = nc.dram_tensor("wk", [dim, HD], bf16, kind="ExternalInput")
    wv = nc.dram_tensor("wv", [dim, HD], bf16, kind="ExternalInput")
    wo = nc.dram_tensor("wo", [HD, dim], bf16, kind="ExternalInput")
    bq2 = nc.dram_tensor("bq2", [128, 2], f32, kind="ExternalInput")
    bk2 = nc.dram_tensor("bk2", [128, 2], f32, kind="ExternalInput")
    bv = nc.dram_tensor("bv", [1, HD], f32, kind="ExternalInput")
    tri = nc.dram_tensor("tri", [128, 128], bf16, kind="ExternalInput")
    out = nc.dram_tensor("out", [n, dim], f32, kind="ExternalOutput")
    with tile.TileContext(nc) as tc:
        _emit(tc, xT.ap(), wq.ap(), wk.ap(), wv.ap(), wo.ap(), bq2.ap(),
              bk2.ap(), bv.ap(), tri.ap(), out.ap(), n, dim)
    nc.finalize()
    return nc


_NC = None


def _get_nc():
    global _NC
    if _NC is None:
        _NC = build()
    return _NC


def make_in_maps(x, Wq, bq, Wkv, bkv, Wo):
    tri = np.triu(np.ones((128, 128), np.float32)).astype(BF16)
    xts = [np.ascontiguousarray(x[b].T).astype(BF16) for b in range(B)]
    in_maps = []
    for c in range(NCORES):
        b, g = divmod(c, NCORES // B)
        cs = slice(HD * g, HD * (g + 1))
        in_maps.append({
            "xT": xts[b],
            "wq": np.ascontiguousarray(Wq[:, cs]).astype(BF16),
            "wk": np.ascontiguousarray(Wkv[:, HD * g:HD * (g + 1)]).astype(BF16),
            "wv": np.ascontiguousarray(Wkv[:, DIM + HD * g:DIM + HD * (g + 1)]).astype(BF16),
            "wo": np.ascontiguousarray(Wo[cs, :]).astype(BF16),
            "bq2": np.ascontiguousarray(bq[cs].reshape(2, 128).T).astype(np.float32),
            "bk2": np.ascontiguousarray(bkv[HD * g:HD * (g + 1)].reshape(2, 128).T).astype(np.float32),
            "bv": np.ascontiguousarray(bkv[DIM + HD * g:DIM + HD * (g + 1)].reshape(1, HD)).astype(np.float32),
            "tri": tri,
        })
    return in_maps


def _run(x, Wq, bq, Wkv, bkv, Wo, bo, **spmd_kwargs):
    x = np.asarray(x, np.float32)
    Wq = np.asarray(Wq, np.float32)
    bq = np.asarray(bq, np.float32)
    Wkv = np.asarray(Wkv, np.float32)
    bkv = np.asarray(bkv, np.float32)
    Wo = np.asarray(Wo, np.float32)
    bo = np.asarray(bo, np.float32)
    nc = _get_nc()
    in_maps = make_in_maps(x, Wq, bq, Wkv, bkv, Wo)
    res = run_bass_kernel_spmd(nc, in_maps, core_ids=list(range(NCORES)),
                               **spmd_kwargs)
    g = NCORES // B
    y = np.empty((B, N, DIM), np.float32)
    for b in range(B):
        acc = res.results[g * b]["out"].astype(np.float32)
        for i in range(1, g):
            acc = acc + res.results[g * b + i]["out"]
        y[b] = acc + bo
    return y, res


def kernel(x, Wq, bq, Wkv, bkv, Wo, bo):
    return _run(x, Wq, bq, Wkv, bkv, Wo, bo)[0]
